# revision 14
# baseline (speedup 1.0000x reference)
"""DeepseekV3 MLA flash-attention prefill kernel for 8 Trainium2 NeuronCores.

Sharding strategy (SPMD, one program for all 8 cores):
  Stage A (sequence-parallel, feature-major): core c computes the low-rank
    down-projections for its 256 rows in transposed layout (lhsT = weight
    chunks, rhs = X^T).  Everything ships RAW: kv chunks + k_pe + the kv
    sum-of-squares row go out in one early AllGather; the 12 raw qa chunks
    + q sum-of-squares row go out in TWO AllGathers (8 chunks + 4 chunks)
    so stage B can start the q up-projection on the first half while the
    second half is still in flight.  No rope / rms work in stage A at all.
  Stage B (head-parallel): core c owns heads {2c, 2c+1}.  While the AGs
    fly it preloads weights and host-precomputed cos/sin tables.  After
    AG-kv: rms-scale the gathered ckv in place, rope k_pe, build K^T / V.
    After AG-q: per-panel q projections (Wqb + RoPE, rms scale folded at
    eviction), attention head 0 interleaved per panel, then AllToAll h0
    overlapped with attention head 1, AllToAll h1 overlapped with the
    first Wo pass (even heads); second Wo pass (odd heads) after.
    Causal attention runs in (k, q) layout: softmax without
    max-subtraction, fully-masked k-blocks skipped, diagonal blocks
    column-trimmed (matmul+exp only on the live slice, GpSimd
    affine_select zero-fills the rest).  Attention output is shipped
    UNNORMALIZED through a per-head AllToAll together with the f32
    softmax sums (bitcast rows); normalization happens after the
    exchange.  Host concatenates the per-core outputs.
"""

import sys

if '/opt/trn_rl_repo' not in sys.path:
    sys.path.insert(0, '/opt/trn_rl_repo')

import numpy as np
import ml_dtypes

import concourse.bass as bass
import concourse.mybir as mybir
import concourse.tile as tile
from concourse import bacc
from concourse.bass_utils import run_bass_kernel_spmd

f32 = mybir.dt.float32
f32r = mybir.dt.float32r
bf16 = mybir.dt.bfloat16
i32 = mybir.dt.int32
AF = mybir.ActivationFunctionType
ALU = mybir.AluOpType

NC_ = 8            # cores
S = 2048           # sequence
HID = 2048
QLR = 1536         # q lora rank
KVLR = 512         # kv lora rank
ROPE = 64
NOPE = 128
VD = 128
NH = 16
HPC = NH // NC_    # heads per core = 2
SL = S // NC_      # rows per core = 256
PANEL = 512        # q panel width
NPANEL = S // PANEL
NKB = S // 128     # 16 k blocks
QCH = QLR // 128   # 12
QACH = 8           # q chunks in first AllGather
QBCH = QCH - QACH  # 4
KCH = KVLR // 128  # 4
HCH = HID // 128   # 16
THETA = 10000.0
SM_SCALE = float((NOPE + ROPE) ** -0.5)
EPS = 1e-6

DT = bf16          # matmul dtype

# ag_in_kv row map: [0:512) raw kv chunks, [512:576) raw k_pe,
# [576:578) kv ssq row (f32 bitcast).
KVROWS = KCH * 128 + ROPE + 2                # 578
QAROWS = QACH * 128                          # 1024
QBROWS = QBCH * 128 + 2                      # 514 (last 2: q ssq bitcast)
# a2a shard: 128 attn rows + 2 rows of f32-bitcast softmax sums.
SHARD = VD + 2                               # 130

_CACHE = {}


def build_program(dt):
    nc = bacc.Bacc("TRN2", target_bir_lowering=False, debug=False, num_devices=NC_)

    def din(name, shape):
        return nc.dram_tensor(name, shape, dt, kind="ExternalInput")

    # ---- external I/O (per-core data) ----
    x_t = din("x_t", [HID, SL])                 # X rows, transposed (hid-major)
    wa_kv = din("wa_kv", [HID, KVLR + ROPE])    # [Wkva(kv) | Wkva(pe, deint)]
    wa_q = din("wa_q", [HID, QLR])              # Wqa
    wqb = din("wqb", [QLR, HPC * 256])          # [nope|pe_d|rot] per head
    wkvb_k = din("wkvb_k", [KVLR, HPC * NOPE])
    wkvb_v = din("wkvb_v", [KVLR, HPC * VD])
    wo = din("wo", [NH * VD, HID])
    ones_col = din("ones_col", [128, 1])
    sel_in = nc.dram_tensor("sel_in", [NC_, NC_ * 128], f32, kind="ExternalInput")
    cos_in = din("cos_in", [ROPE, S])
    sin_in = din("sin_in", [ROPE, S])
    out_loc = nc.dram_tensor("out_loc", [SL, HID], f32, kind="ExternalOutput")

    with tile.TileContext(nc) as tc:
        with tc.tile_pool(name="dram", bufs=1, space="DRAM") as dpool, \
             tc.tile_pool(name="consts", bufs=1) as cpool:
            ag_in_kv = dpool.tile([KVROWS, SL], dt)
            ag_out_kv = dpool.tile([NC_ * KVROWS, SL], dt, addr_space="Shared")
            ag_in_qa = dpool.tile([QAROWS, SL], dt)
            ag_out_qa = dpool.tile([NC_ * QAROWS, SL], dt, addr_space="Shared")
            ag_in_qb = dpool.tile([QBROWS, SL], dt)
            ag_out_qb = dpool.tile([NC_ * QBROWS, SL], dt, addr_space="Shared")
            a2a_in = [dpool.tile([NC_ * SHARD, SL], dt, name=f"a2a_in{h}") for h in range(HPC)]
            a2a_out = [dpool.tile([NC_ * SHARD, SL], dt, name=f"a2a_out{h}") for h in range(HPC)]

            ocol = cpool.tile([128, 1], dt)
            sel = cpool.tile([NC_, NC_ * 128], f32r)
            cos_g = cpool.tile([ROPE, S], dt)
            sin_g = cpool.tile([ROPE, S], dt)
            nc.sync.dma_start(out=ocol[:], in_=ones_col[:])
            nc.sync.dma_start(out=sel[:], in_=sel_in[:].bitcast(f32r))
            nc.gpsimd.dma_start(out=cos_g[:], in_=cos_in[:])
            nc.gpsimd.dma_start(out=sin_g[:], in_=sin_in[:])

            # ================= Stage A: transposed down projections =================
            with tc.tile_pool(name="sa_x", bufs=1) as xp, \
                 tc.tile_pool(name="sa_w", bufs=1) as wp, \
                 tc.tile_pool(name="sa_tmp", bufs=2) as tp, \
                 tc.tile_pool(name="sa_ps", bufs=2, space="PSUM") as pp_a, \
                 tc.tile_pool(name="sa_ps1", bufs=1, space="PSUM") as pp1_a:

                # interleave x / wa_kv chunk loads so the kv matmuls start early
                xts = []
                wkv_res = []
                for k in range(HCH):
                    xt = xp.tile([128, SL], dt, name=f"xt{k}")
                    nc.sync.dma_start(out=xt[:], in_=x_t[128 * k:128 * (k + 1), :])
                    xts.append(xt)
                    wt = wp.tile([128, KVLR + ROPE], dt, name=f"wAkv_{k}")
                    nc.sync.dma_start(out=wt[:], in_=wa_kv[128 * k:128 * (k + 1), :])
                    wkv_res.append(wt)
                wq_res = []
                for k in range(HCH):
                    wt = wp.tile([128, QLR], dt, name=f"wAq_{k}")
                    nc.gpsimd.dma_start(out=wt[:], in_=wa_q[128 * k:128 * (k + 1), :])
                    wq_res.append(wt)

                def a_chunk(wts, o, c0, width, tag):
                    ps = pp_a.tile([width, SL], f32, name=f"ps_{tag}_{o}", tag="a_ps", bufs=2)
                    for hc in range(HCH):
                        nc.tensor.matmul(ps[:], wts[hc][:, c0:c0 + width], xts[hc][:],
                                         start=(hc == 0), stop=(hc == HCH - 1))
                    return ps

                agkv_w = ag_in_kv.rearrange("(a b) q -> a (b q)", b=2)

                # ---- kv chunks (raw) + k_pe (raw) + ssq row ----
                ssq_kv = pp1_a.tile([1, SL], f32, name="ssq_kv")
                for o in range(KCH):
                    ps = a_chunk(wkv_res, o, 128 * o, 128, "kv")
                    sb = tp.tile([128, SL], dt, name=f"kv_sb{o}", tag="kv_sb", bufs=3)
                    nc.vector.tensor_copy(sb[:], ps[:])
                    nc.sync.dma_start(out=ag_in_kv[128 * o:128 * (o + 1), :], in_=sb[:])
                    sq = tp.tile([128, SL], dt, name=f"sqk{o}", tag="sq", bufs=2)
                    nc.scalar.activation(sq[:], ps[:], AF.Square)
                    nc.tensor.matmul(ssq_kv[:], ocol[:], sq[:], start=(o == 0), stop=(o == KCH - 1))
                ps_pe = a_chunk(wkv_res, 0, KVLR, ROPE, "pe")
                pe_sb = tp.tile([ROPE, SL], dt, name="pe_sb", tag="pe_sb", bufs=1)
                nc.vector.tensor_copy(pe_sb[:], ps_pe[:])
                nc.sync.dma_start(out=ag_in_kv[KCH * 128:KCH * 128 + ROPE, :], in_=pe_sb[:])
                sskv_sb = tp.tile([1, SL], f32, name="sskv_sb", tag="ss_sb", bufs=2)
                nc.vector.tensor_copy(sskv_sb[:], ssq_kv[:])
                nc.sync.dma_start(out=agkv_w[(KCH * 128 + ROPE) // 2:(KCH * 128 + ROPE) // 2 + 1, :],
                                  in_=sskv_sb[:].bitcast(dt))

                nc.gpsimd.collective_compute(
                    "AllGather", ALU.bypass,
                    replica_groups=[list(range(NC_))],
                    ins=[ag_in_kv[:]], outs=[ag_out_kv[:]],
                )

                # ---- q chunks: raw, in two AllGathers ----
                ssq_q = pp1_a.tile([1, SL], f32, name="ssq_q")
                for o in range(QCH):
                    ps = a_chunk(wq_res, o, 128 * o, 128, "q")
                    qa_c = tp.tile([128, SL], dt, name=f"qac{o}", tag="qa_c", bufs=3)
                    nc.vector.tensor_copy(qa_c[:], ps[:])
                    if o < QACH:
                        nc.sync.dma_start(out=ag_in_qa[128 * o:128 * (o + 1), :], in_=qa_c[:])
                    else:
                        nc.sync.dma_start(out=ag_in_qb[128 * (o - QACH):128 * (o - QACH + 1), :],
                                          in_=qa_c[:])
                    sq = tp.tile([128, SL], dt, name=f"sqq{o}", tag="sq", bufs=2)
                    nc.scalar.activation(sq[:], ps[:], AF.Square)
                    nc.tensor.matmul(ssq_q[:], ocol[:], sq[:], start=(o == 0), stop=(o == QCH - 1))
                    if o == QACH - 1:
                        nc.gpsimd.collective_compute(
                            "AllGather", ALU.bypass,
                            replica_groups=[list(range(NC_))],
                            ins=[ag_in_qa[:]], outs=[ag_out_qa[:]],
                        )
                ssq_sb = tp.tile([1, SL], f32, name="ssq_sb", tag="ss_sb", bufs=2)
                nc.vector.tensor_copy(ssq_sb[:], ssq_q[:])
                agqb_w = ag_in_qb.rearrange("(a b) q -> a (b q)", b=2)
                nc.sync.dma_start(out=agqb_w[(QBCH * 128) // 2:(QBCH * 128) // 2 + 1, :],
                                  in_=ssq_sb[:].bitcast(dt))

                nc.gpsimd.collective_compute(
                    "AllGather", ALU.bypass,
                    replica_groups=[list(range(NC_))],
                    ins=[ag_in_qb[:]], outs=[ag_out_qb[:]],
                )

            # gathered views: [row, rank, token]
            agkv_3 = ag_out_kv.rearrange("(r c) q -> c r q", r=NC_)
            agkv_r = ag_out_kv.rearrange("(r c) q -> r (c q)", r=NC_)
            agqa_3 = ag_out_qa.rearrange("(r c) q -> c r q", r=NC_)
            agqb_3 = ag_out_qb.rearrange("(r c) q -> c r q", r=NC_)
            agqb_r = ag_out_qb.rearrange("(r c) q -> r (c q)", r=NC_)

            def load_gathered(t, src3, r0, r1):
                """t[p, 256r+q] = src3[r0+p, r, q] in one strided DMA."""
                nc.sync.dma_start(
                    out=t[:].rearrange("p (r q) -> p r q", r=NC_),
                    in_=src3[r0:r1])

            # ================= Stage B: head-parallel attention =================
            with tc.tile_pool(name="sb_res", bufs=1) as rp, \
                 tc.tile_pool(name="sb_tmp", bufs=2) as tp, \
                 tc.tile_pool(name="sb_pt", bufs=4) as ptp, \
                 tc.tile_pool(name="sb_wo", bufs=1) as wsp, \
                 tc.tile_pool(name="sb_ag", bufs=1) as agp:

                # weight preloads (gpsimd queue; scalar stays free for exp)
                wkk_t = []
                wkv_t = []
                for l in range(KCH):
                    t = rp.tile([128, HPC * NOPE], dt, name=f"wkk_t{l}")
                    nc.gpsimd.dma_start(out=t[:], in_=wkvb_k[128 * l:128 * (l + 1), :])
                    wkk_t.append(t)
                    t2 = rp.tile([128, HPC * VD], dt, name=f"wkv_t{l}")
                    nc.gpsimd.dma_start(out=t2[:], in_=wkvb_v[128 * l:128 * (l + 1), :])
                    wkv_t.append(t2)
                # wo preload (gpsimd queue, overlaps the AllGathers / attention)
                wo_t = []
                for c in range(HCH):
                    t = wsp.tile([128, HID], dt, name=f"wo_t{c}")
                    nc.gpsimd.dma_start(out=t[:], in_=wo[128 * c:128 * (c + 1), :])
                    wo_t.append(t)

                # ---- post-AG-kv: rms scale, k_pe rope, K^T, V ----
                kT = [rp.tile([128, S], dt, name=f"kT{h}") for h in range(HPC)]
                v_t = [rp.tile([128, HPC * VD], dt, name=f"v_t{kb}") for kb in range(NKB)]
                kpe_f = rp.tile([ROPE, S], dt, name="kpe_f")
                with tc.tile_pool(name="sb_ckv", bufs=1) as ckvp, \
                     tc.tile_pool(name="sb_eps", bufs=2, space="PSUM") as pp_e:
                    ckv_g = []
                    for j in range(KCH):
                        t = ckvp.tile([128, S], dt, name=f"ckv_g{j}")
                        load_gathered(t, agkv_3, 128 * j, 128 * (j + 1))
                        ckv_g.append(t)
                    with tc.tile_pool(name="sb_kpe", bufs=1) as kpp:
                        kpe_b = kpp.tile([ROPE, S], dt, name="kpe_b")
                        load_gathered(kpe_b, agkv_3, KCH * 128, KCH * 128 + ROPE)
                        sskv_b = kpp.tile([NC_, 2 * SL], dt, name="sskv_b")
                        nc.sync.dma_start(
                            out=sskv_b[:],
                            in_=agkv_r[:, (KCH * 128 + ROPE) * SL:KVROWS * SL])

                        # rkv broadcast [128, S]
                        ms_kv = kpp.tile([NC_, SL], f32, name="ms_kv")
                        nc.scalar.activation(ms_kv[:], sskv_b[:].bitcast(f32), AF.Sqrt,
                                             scale=1.0 / KVLR)
                        rkv8 = kpp.tile([NC_, SL], f32r, name="rkv8")
                        with nc.allow_low_precision(reason="f32r rounding of rms scale"):
                            nc.vector.reciprocal(rkv8[:], ms_kv[:])
                        rkv_sb = kpp.tile([128, S], dt, name="rkv_sb")
                        for half in range(2):
                            bc = pp_e.tile([128, S // 2], f32, name=f"bckv{half}", tag="bc", bufs=2)
                            for m in range(4):
                                r = 4 * half + m
                                nc.tensor.matmul(bc[:, SL * m:SL * (m + 1)],
                                                 sel[:, 128 * r:128 * (r + 1)], rkv8[:],
                                                 start=True, stop=True)
                            nc.vector.tensor_copy(rkv_sb[:, (S // 2) * half:(S // 2) * (half + 1)], bc[:])
                        # scale ckv in place
                        for j in range(KCH):
                            nc.vector.tensor_mul(ckv_g[j][:], ckv_g[j][:], rkv_sb[:])

                        # k_pe rope (local bf16 cos/sin tables)
                        krot = kpp.tile([ROPE, S], dt, name="krot")
                        nc.vector.tensor_scalar(out=krot[0:32, :], in0=kpe_b[32:64, :],
                                                scalar1=-1.0, scalar2=None, op0=ALU.mult)
                        nc.vector.tensor_copy(krot[32:64, :], kpe_b[0:32, :])
                        kro = kpp.tile([ROPE, S], dt, name="kro")
                        nc.vector.tensor_mul(kro[:], kpe_b[:], cos_g[:])
                        nc.vector.tensor_mul(krot[:], krot[:], sin_g[:])
                        nc.vector.tensor_add(kpe_f[:], kro[:], krot[:])

                    # K^T and V (both heads)
                    for h in range(HPC):
                        for kc in range(S // 512):
                            ps = pp_e.tile([128, 512], f32, name=f"kt_ps{h}_{kc}", tag="bc", bufs=2)
                            for l in range(KCH):
                                nc.tensor.matmul(ps[:], wkk_t[l][:, NOPE * h:NOPE * (h + 1)],
                                                 ckv_g[l][:, 512 * kc:512 * (kc + 1)],
                                                 start=(l == 0), stop=(l == KCH - 1))
                            nc.vector.tensor_copy(kT[h][:, 512 * kc:512 * (kc + 1)], ps[:])
                    for kb in range(NKB):
                        ps = pp_e.tile([128, HPC * VD], f32, name=f"v_ps{kb}", tag="bc", bufs=2)
                        for l in range(KCH):
                            nc.tensor.matmul(ps[:], ckv_g[l][:, 128 * kb:128 * (kb + 1)], wkv_t[l][:],
                                             start=(l == 0), stop=(l == KCH - 1))
                        nc.vector.tensor_copy(v_t[kb][:], ps[:])

                # ---- q projections + attention h0, interleaved per panel ----
                with tc.tile_pool(name="sb_qps", bufs=1, space="PSUM") as qpp, \
                     tc.tile_pool(name="sb_rq", bufs=1, space="PSUM") as rqp, \
                     tc.tile_pool(name="sb_sc", bufs=2, space="PSUM") as scp, \
                     tc.tile_pool(name="sb_at", bufs=2, space="PSUM") as atp, \
                     tc.tile_pool(name="sb_sum", bufs=1, space="PSUM") as sup, \
                     tc.tile_pool(name="sb_qa", bufs=2) as qap:

                    wqb_t = []
                    for l in range(QCH):
                        t = qap.tile([128, HPC * 256], dt, name=f"wqb_t{l}",
                                     tag=f"wqb{l}", bufs=1)
                        nc.gpsimd.dma_start(out=t[:], in_=wqb[128 * l:128 * (l + 1), :])
                        wqb_t.append(t)

                    # q rms scale row (needs AG qb; scalar queue so the sync
                    # queue's qa panel loads aren't blocked behind it)
                    ssq8_b = qap.tile([NC_, 2 * SL], dt, name="ssq8_b", tag="ssq8", bufs=1)
                    nc.scalar.dma_start(out=ssq8_b[:],
                                        in_=agqb_r[:, QBCH * 128 * SL:QBROWS * SL])
                    ms8 = qap.tile([NC_, SL], f32, name="ms8", tag="ms8", bufs=1)
                    nc.scalar.activation(ms8[:], ssq8_b[:].bitcast(f32), AF.Sqrt,
                                         scale=1.0 / QLR)
                    rq8_t = qap.tile([NC_, SL], f32r, name="rq8_t", tag="rq8", bufs=1)
                    with nc.allow_low_precision(reason="f32r rounding of rms scale"):
                        nc.vector.reciprocal(rq8_t[:], ms8[:])
                    rq8 = rq8_t[:]

                    qn_sb = {}
                    qp_sb = {}

                    def q_panel(p, h):
                        """q projections for (head h, panel p); rms scale folded."""
                        qs = slice(PANEL * p, PANEL * (p + 1))
                        if h == 0:
                            qa_p = []
                            for l in range(QCH):
                                t = qap.tile([128, PANEL], dt, name=f"qa_p{p}_{l}",
                                             tag=f"qa_p{l}", bufs=2)
                                # AG-qb-dependent loads go on the scalar queue so
                                # they don't block AG-qa loads on the sync queue
                                src3 = agqa_3 if l < QACH else agqb_3
                                r0 = 128 * l if l < QACH else 128 * (l - QACH)
                                eng = nc.sync if l < QACH else nc.scalar
                                eng.dma_start(
                                    out=t[:].rearrange("p (r q) -> p r q", r=2),
                                    in_=src3[r0:r0 + 128, 2 * p:2 * p + 2])
                                qa_p.append(t)
                            q_panel.qa = qa_p
                        qa_p = q_panel.qa
                        hcol = 256 * h
                        ps_qn = qpp.tile([128, PANEL], f32, name=f"qn_ps{h}_{p}", tag="qn", bufs=1)
                        for l in range(QCH):
                            nc.tensor.matmul(ps_qn[:], wqb_t[l][:, hcol:hcol + NOPE], qa_p[l][:],
                                             start=(l == 0), stop=(l == QCH - 1))
                        ps_qr = qpp.tile([128, PANEL], f32, name=f"qr_ps{h}_{p}", tag="qr", bufs=1)
                        for l in range(QCH):
                            nc.tensor.matmul(ps_qr[:], wqb_t[l][:, hcol + NOPE:hcol + 256], qa_p[l][:],
                                             start=(l == 0), stop=(l == QCH - 1))
                        if h == 0:
                            rqbc = rqp.tile([128, PANEL], f32, name=f"rqbc{p}", tag="rqbc", bufs=1)
                            nc.tensor.matmul(rqbc[:, 0:SL], sel[:, 128 * (2 * p):128 * (2 * p + 1)],
                                             rq8, start=True, stop=True)
                            nc.tensor.matmul(rqbc[:, SL:PANEL], sel[:, 128 * (2 * p + 1):128 * (2 * p + 2)],
                                             rq8, start=True, stop=True)
                            rqbc_sb = tp.tile([128, PANEL], f32, name=f"rqbc_sb{p}", tag="rqbc_sb", bufs=2)
                            nc.vector.tensor_copy(rqbc_sb[:], rqbc[:])
                            q_panel.rq = rqbc_sb
                            cos_s = tp.tile([ROPE, PANEL], dt, name=f"cos_s{p}", tag="cos_s", bufs=2)
                            nc.vector.tensor_mul(cos_s[:], cos_g[:, qs], rqbc_sb[0:ROPE, :])
                            sin_s = tp.tile([ROPE, PANEL], dt, name=f"sin_s{p}", tag="sin_s", bufs=2)
                            nc.vector.tensor_mul(sin_s[:], sin_g[:, qs], rqbc_sb[0:ROPE, :])
                            q_panel.cs = (cos_s, sin_s)
                        rqbc_sb = q_panel.rq
                        cos_s, sin_s = q_panel.cs
                        qn = rp.tile([128, PANEL], dt, name=f"qn_sb{h}_{p}")
                        nc.vector.tensor_mul(qn[:], ps_qn[:], rqbc_sb[:])
                        qn_sb[(h, p)] = qn
                        qt1 = tp.tile([ROPE, PANEL], f32, name=f"qt1_{h}_{p}", tag="qt1", bufs=1)
                        nc.vector.tensor_mul(qt1[:], ps_qr[0:ROPE, :], cos_s[:])
                        qt2 = tp.tile([ROPE, PANEL], f32, name=f"qt2_{h}_{p}", tag="qt2", bufs=1)
                        nc.vector.tensor_mul(qt2[:], ps_qr[ROPE:2 * ROPE, :], sin_s[:])
                        qp = rp.tile([ROPE, PANEL], dt, name=f"qp_sb{h}_{p}")
                        nc.vector.tensor_add(qp[:], qt1[:], qt2[:])
                        qp_sb[(h, p)] = qp

                    a2a_vx = [a2a_in[h].rearrange("(j x) q -> x j q", x=SHARD) for h in range(HPC)]
                    a2a_w = [a2a_in[h].rearrange("(a b) q -> a (b q)", b=2) for h in range(HPC)]
                    a2a_wv = [a2a_w[h].rearrange("(j y) q -> y j q", y=SHARD // 2) for h in range(HPC)]

                    def attn_panel(h, p):
                        """causal attention for (head h, panel p), unnormalized."""
                        nkb = 4 * (p + 1)
                        ps_at = atp.tile([128, PANEL], f32, name=f"at_ps{h}_{p}", tag="at_ps", bufs=2)
                        ps_sum = sup.tile([1, PANEL], f32, name=f"sum_ps{h}_{p}", tag="sum_ps", bufs=1)
                        pts = {}
                        np_ = nkb // 2  # sum pairs

                        def consume(kb):
                            nc.tensor.matmul(ps_at[:], v_t[kb][:, VD * h:VD * (h + 1)], pts[kb][:],
                                             start=(kb == 0), stop=(kb == nkb - 1))
                            if kb % 2 == 1:
                                pr = kb // 2
                                padd = tp.tile([128, PANEL], dt, name=f"padd{h}_{p}_{kb}",
                                               tag="padd", bufs=2)
                                nc.vector.tensor_add(padd[:], pts[kb - 1][:], pts[kb][:])
                                nc.tensor.matmul(ps_sum[:], ocol[:], padd[:],
                                                 start=(pr == 0), stop=(pr == np_ - 1))

                        for kb in range(nkb):
                            j = kb - 4 * p  # >= 0 on diagonal blocks
                            c0 = 128 * j if j > 0 else 0
                            ps_sc = scp.tile([128, PANEL], f32, name=f"sc_ps{h}_{p}_{kb}",
                                             tag="sc_ps", bufs=2)
                            nc.tensor.matmul(ps_sc[:, c0:], kT[h][:, 128 * kb:128 * (kb + 1)],
                                             qn_sb[(h, p)][:, c0:], start=True, stop=False)
                            nc.tensor.matmul(ps_sc[:, c0:], kpe_f[:, 128 * kb:128 * (kb + 1)],
                                             qp_sb[(h, p)][:, c0:], start=False, stop=True)
                            pt = ptp.tile([128, PANEL], dt, name=f"pt{h}_{p}_{kb}", tag="pt", bufs=4)
                            nc.scalar.activation(pt[:, c0:], ps_sc[:, c0:], AF.Exp, scale=SM_SCALE)
                            if j >= 0:
                                nc.gpsimd.affine_select(
                                    out=pt[:], in_=pt[:],
                                    pattern=[[1, PANEL]],
                                    compare_op=ALU.is_ge,
                                    fill=0.0,
                                    base=-128 * j,
                                    channel_multiplier=-1)
                            pts[kb] = pt
                            if kb > 0:
                                consume(kb - 1)
                        consume(nkb - 1)
                        at_b = tp.tile([128, PANEL], dt, name=f"at_b{h}_{p}", tag="at_b", bufs=2)
                        nc.vector.tensor_copy(at_b[:], ps_at[:])
                        sum_sb = tp.tile([1, PANEL], f32, name=f"sum_sb{h}_{p}", tag="sum_sb", bufs=2)
                        nc.vector.tensor_copy(sum_sb[:], ps_sum[:])
                        nc.sync.dma_start(
                            out=a2a_vx[h][0:VD, 2 * p:2 * p + 2],
                            in_=at_b[:].rearrange("p (r q) -> p r q", r=2))
                        nc.sync.dma_start(
                            out=a2a_wv[h][VD // 2:VD // 2 + 1, 2 * p:2 * p + 2],
                            in_=sum_sb[:].bitcast(dt).rearrange("p (r q) -> p r q", r=2))

                    for p in range(NPANEL):
                        q_panel(p, 0)
                        q_panel(p, 1)
                        attn_panel(0, p)
                    nc.gpsimd.collective_compute(
                        "AllToAll", ALU.bypass,
                        replica_groups=[list(range(NC_))],
                        ins=[a2a_in[0][:]], outs=[a2a_out[0][:]],
                    )
                    for p in range(NPANEL):
                        attn_panel(1, p)
                    nc.gpsimd.collective_compute(
                        "AllToAll", ALU.bypass,
                        replica_groups=[list(range(NC_))],
                        ins=[a2a_in[1][:]], outs=[a2a_out[1][:]],
                    )

                    # ---- post-exchange normalization + two-pass Wo ----
                    osb_stash = {}
                    for h in range(HPC):
                        a2a_ox = a2a_out[h].rearrange("(j x) q -> x j q", x=SHARD)
                        a2a_oj = a2a_out[h].rearrange("(j x) q -> j (x q)", x=SHARD)
                        att_all = agp.tile([128, NC_ * SL], dt, name=f"att_all{h}")
                        nc.sync.dma_start(out=att_all[:].rearrange("p (r q) -> p r q", r=NC_),
                                          in_=a2a_ox[0:VD])
                        sums_b = agp.tile([NC_, 2 * SL], dt, name=f"sums_b{h}")
                        nc.sync.dma_start(out=sums_b[:], in_=a2a_oj[:, VD * SL:SHARD * SL])
                        rec8 = agp.tile([NC_, SL], f32r, name=f"rec8{h}")
                        with nc.allow_low_precision(reason="f32r rounding of softmax recip"):
                            nc.vector.reciprocal(rec8[:], sums_b[:].bitcast(f32))
                        for m in range(NC_):
                            bc = atp.tile([128, SL], f32, name=f"nbc{h}_{m}", tag="at_ps", bufs=2)
                            nc.tensor.matmul(bc[:], sel[:, 128 * m:128 * (m + 1)], rec8[:],
                                             start=True, stop=True)
                            nc.vector.tensor_mul(att_all[:, SL * m:SL * (m + 1)],
                                                 att_all[:, SL * m:SL * (m + 1)], bc[:])
                        # Wo pass h: accumulate this parity's 8 head-chunks
                        for col in range(HID // 512):
                            for qb in range(SL // 128):
                                ps = qpp.tile([128, 512], f32, name=f"o_ps{h}_{col}_{qb}",
                                              tag=("qn" if (col + qb) % 2 == 0 else "qr"), bufs=1)
                                for m in range(NC_):
                                    nc.tensor.matmul(ps[:], att_all[:, SL * m + 128 * qb:SL * m + 128 * (qb + 1)],
                                                     wo_t[2 * m + h][:, 512 * col:512 * (col + 1)],
                                                     start=(m == 0), stop=(m == NC_ - 1))
                                if h == 0:
                                    st = wsp.tile([128, 512], dt, name=f"ost{col}_{qb}")
                                    nc.vector.tensor_copy(st[:], ps[:])
                                    osb_stash[(col, qb)] = st
                                else:
                                    osb = tp.tile([128, 512], f32, name=f"osb{col}_{qb}", tag="osb", bufs=2)
                                    nc.vector.tensor_add(osb[:], ps[:], osb_stash[(col, qb)][:])
                                    nc.sync.dma_start(out=out_loc[128 * qb:128 * (qb + 1), 512 * col:512 * (col + 1)],
                                                      in_=osb[:])

    nc.compile()
    return nc


def _to_dt(a, dt):
    if dt == bf16:
        return np.ascontiguousarray(a.astype(ml_dtypes.bfloat16))
    return np.ascontiguousarray(a.astype(np.float32))


def _prepare_inputs(dt, hidden_states, position_ids, Wqa, qa_ln_w, Wqb, Wkva, kv_ln_w, Wkvb, Wo):
    perm = np.concatenate([np.arange(0, ROPE, 2), np.arange(1, ROPE, 2)])
    X = np.asarray(hidden_states, np.float32).reshape(S, HID)
    Wqa = np.asarray(Wqa, np.float32)
    Wkva = np.asarray(Wkva, np.float32)
    wa_kv = np.concatenate([Wkva[:, :KVLR], Wkva[:, KVLR:][:, perm]], axis=1)  # (2048, 576)
    wqb_base = np.asarray(Wqb, np.float32) * np.asarray(qa_ln_w, np.float32)[:, None]
    wkvb_base = np.asarray(Wkvb, np.float32) * np.asarray(kv_ln_w, np.float32)[:, None]
    Wo = np.asarray(Wo, np.float32)

    head_blocks = []
    for h in range(NH):
        cols = wqb_base[:, 192 * h:192 * (h + 1)]
        nope = cols[:, :NOPE]
        pe_d = cols[:, NOPE:][:, perm]
        rot = np.concatenate([-pe_d[:, 32:], pe_d[:, :32]], axis=1)
        head_blocks.append(np.concatenate([nope, pe_d, rot], axis=1))  # (1536, 256)
    k_blocks = [wkvb_base[:, 256 * h:256 * h + NOPE] for h in range(NH)]
    v_blocks = [wkvb_base[:, 256 * h + NOPE:256 * (h + 1)] for h in range(NH)]

    # host-precomputed rope tables (deinterleaved layout), [64, S] transposed
    inv = (1.0 / (THETA ** (np.arange(0, ROPE, 2, dtype=np.float64) / ROPE)))
    pos = np.asarray(position_ids).reshape(-1).astype(np.float64)
    emb = np.concatenate([np.outer(pos, inv), np.outer(pos, inv)], axis=1)  # (S, 64)
    cos_np = _to_dt(np.cos(emb).T.astype(np.float32), dt)
    sin_np = _to_dt(np.sin(emb).T.astype(np.float32), dt)

    wakv_d = _to_dt(wa_kv, dt)
    waq_d = _to_dt(Wqa, dt)
    wo_d = _to_dt(Wo, dt)
    ones_col_d = _to_dt(np.ones((128, 1), np.float32), dt)
    sel_np = np.zeros((NC_, NC_ * 128), np.float32)
    for m in range(NC_):
        sel_np[m, 128 * m:128 * (m + 1)] = 1.0

    in_maps = []
    for c in range(NC_):
        rows = slice(SL * c, SL * (c + 1))
        in_maps.append({
            "x_t": _to_dt(X[rows, :].T, dt),
            "wa_kv": wakv_d,
            "wa_q": waq_d,
            "wqb": _to_dt(np.concatenate([head_blocks[HPC * c + h] for h in range(HPC)], axis=1), dt),
            "wkvb_k": _to_dt(np.concatenate([k_blocks[HPC * c + h] for h in range(HPC)], axis=1), dt),
            "wkvb_v": _to_dt(np.concatenate([v_blocks[HPC * c + h] for h in range(HPC)], axis=1), dt),
            "wo": wo_d,
            "ones_col": ones_col_d,
            "sel_in": sel_np,
            "cos_in": cos_np,
            "sin_in": sin_np,
        })
    return in_maps


def run(inputs, trace=False, trace_cores=None, dt=None):
    dt = dt if dt is not None else DT
    key = ("nc", str(dt))
    if key not in _CACHE:
        _CACHE[key] = build_program(dt)
    nc = _CACHE[key]
    in_maps = _prepare_inputs(dt, **inputs)
    res = run_bass_kernel_spmd(nc, in_maps, list(range(NC_)), trace=trace,
                               trace_cores=trace_cores)
    out = np.concatenate([res.results[c]["out_loc"] for c in range(NC_)], axis=0)
    return out.reshape(1, S, HID), res


def kernel(**inputs) -> np.ndarray:
    out, _ = run(inputs, trace=False)
    return out


# revision 23
# speedup vs baseline: 1.0434x; 1.0434x over previous
"""DeepseekV3 MLA flash-attention prefill kernel for 8 Trainium2 NeuronCores.

Sharding strategy (SPMD, one program for all 8 cores):
  Stage A (sequence-parallel, feature-major): core c computes the low-rank
    down-projections for its 256 rows in transposed layout (lhsT = weight
    chunks, rhs = X^T).  Everything ships RAW: kv chunks + k_pe + the kv
    sum-of-squares row go out in one early AllGather; the 12 raw qa chunks
    + q sum-of-squares row go out in TWO AllGathers (8 chunks + 4 chunks)
    so stage B can start the q up-projection on the first half while the
    second half is still in flight.  No rope / rms work in stage A at all.
  Stage B (head-parallel): core c owns heads {2c, 2c+1}.  While the AGs
    fly it preloads weights and host-precomputed cos/sin tables.  After
    AG-kv: rms-scale the gathered ckv in place, rope k_pe, build K^T / V.
    After AG-q: per-panel q projections (Wqb + RoPE, rms scale folded at
    eviction), attention head 0 interleaved per panel, then AllToAll h0
    overlapped with attention head 1, AllToAll h1 overlapped with the
    first Wo pass (even heads); second Wo pass (odd heads) after.
    Causal attention runs in (k, q) layout: softmax without
    max-subtraction, fully-masked k-blocks skipped, diagonal blocks
    column-trimmed (matmul+exp only on the live slice, GpSimd
    affine_select zero-fills the rest).  Attention output is shipped
    UNNORMALIZED through a per-head AllToAll together with the f32
    softmax sums (bitcast rows); normalization happens after the
    exchange.  Host concatenates the per-core outputs.
"""

import sys

if '/opt/trn_rl_repo' not in sys.path:
    sys.path.insert(0, '/opt/trn_rl_repo')

import numpy as np
import ml_dtypes

import concourse.bass as bass
import concourse.mybir as mybir
import concourse.tile as tile
from concourse import bacc
from concourse.bass_utils import run_bass_kernel_spmd

f32 = mybir.dt.float32
f32r = mybir.dt.float32r
bf16 = mybir.dt.bfloat16
i32 = mybir.dt.int32
AF = mybir.ActivationFunctionType
ALU = mybir.AluOpType

NC_ = 8            # cores
S = 2048           # sequence
HID = 2048
QLR = 1536         # q lora rank
KVLR = 512         # kv lora rank
ROPE = 64
NOPE = 128
VD = 128
NH = 16
HPC = NH // NC_    # heads per core = 2
SL = S // NC_      # rows per core = 256
PANEL = 512        # q panel width
NPANEL = S // PANEL
NKB = S // 128     # 16 k blocks
QCH = QLR // 128   # 12
QACH = 8           # q chunks in first AllGather
QBCH = QCH - QACH  # 4
KCH = KVLR // 128  # 4
HCH = HID // 128   # 16
THETA = 10000.0
SM_SCALE = float((NOPE + ROPE) ** -0.5)
EPS = 1e-6

DT = bf16          # matmul dtype

# ag_in_kv row map: [0:512) rms-scaled kv chunks, [512:576) raw k_pe.
KVROWS = KCH * 128 + ROPE                    # 576
# ag_in_q row map: [0:1536) raw qa chunks, [1536:1538) q ssq (f32 bitcast).
QROWS = QCH * 128 + 2                        # 1538
# a2a shard: 128 attn rows + 2 rows of f32-bitcast softmax sums.
SHARD = VD + 2                               # 130

_CACHE = {}


def build_program(dt):
    nc = bacc.Bacc("TRN2", target_bir_lowering=False, debug=False, num_devices=NC_)

    def din(name, shape):
        return nc.dram_tensor(name, shape, dt, kind="ExternalInput")

    # ---- external I/O (per-core data) ----
    x_t = din("x_t", [HID, SL])                 # X rows, transposed (hid-major)
    wa_kv = din("wa_kv", [HID, KVLR + ROPE])    # [Wkva(kv) | Wkva(pe, deint)]
    wa_q = din("wa_q", [HID, QLR])              # Wqa
    wqb = din("wqb", [QLR, HPC * 256])          # [nope|pe_d|rot] per head
    wkvb_k = din("wkvb_k", [KVLR, HPC * NOPE])
    wkvb_v = din("wkvb_v", [KVLR, HPC * VD])
    wo = din("wo", [NH * VD, HID])
    ones_col = din("ones_col", [128, 1])
    ones_row = nc.dram_tensor("ones_row", [1, 128], f32, kind="ExternalInput")
    sel_in = nc.dram_tensor("sel_in", [NC_, NC_ * 128], f32, kind="ExternalInput")
    cos_in = din("cos_in", [ROPE, S])
    sin_in = din("sin_in", [ROPE, S])
    out_loc = nc.dram_tensor("out_loc", [SL, HID], f32, kind="ExternalOutput")

    with tile.TileContext(nc) as tc:
        with tc.tile_pool(name="dram", bufs=1, space="DRAM") as dpool, \
             tc.tile_pool(name="consts", bufs=1) as cpool:
            ag_in_kv = dpool.tile([KVROWS, SL], dt)
            ag_out_kv = dpool.tile([NC_ * KVROWS, SL], dt, addr_space="Shared")
            ag_in_q = dpool.tile([QROWS, SL], dt)
            ag_out_q = dpool.tile([NC_ * QROWS, SL], dt, addr_space="Shared")
            a2a_in = [dpool.tile([NC_ * SHARD, SL], dt, name=f"a2a_in{h}") for h in range(HPC)]
            a2a_out = [dpool.tile([NC_ * SHARD, SL], dt, name=f"a2a_out{h}") for h in range(HPC)]

            ocol = cpool.tile([128, 1], dt)
            orow = cpool.tile([1, 128], f32r)
            sel = cpool.tile([NC_, NC_ * 128], f32r)
            cos_g = cpool.tile([ROPE, S], dt)
            sin_g = cpool.tile([ROPE, S], dt)
            nc.sync.dma_start(out=ocol[:], in_=ones_col[:])
            nc.sync.dma_start(out=orow[:], in_=ones_row[:].bitcast(f32r))
            nc.sync.dma_start(out=sel[:], in_=sel_in[:].bitcast(f32r))
            nc.gpsimd.dma_start(out=cos_g[:], in_=cos_in[:])
            nc.gpsimd.dma_start(out=sin_g[:], in_=sin_in[:])

            # ================= Stage A: transposed down projections =================
            with tc.tile_pool(name="sa_x", bufs=1) as xp, \
                 tc.tile_pool(name="sa_w", bufs=1) as wp, \
                 tc.tile_pool(name="sa_res", bufs=1) as rp_a, \
                 tc.tile_pool(name="sa_tmp", bufs=2) as tp, \
                 tc.tile_pool(name="sa_ps", bufs=2, space="PSUM") as pp_a, \
                 tc.tile_pool(name="sa_ps1", bufs=1, space="PSUM") as pp1_a:

                # spread chunk loads over three queues so no single DMA queue
                # bottlenecks the stage-A ramp
                xts = []
                wkv_res = []
                for k in range(HCH):
                    xt = xp.tile([128, SL], dt, name=f"xt{k}")
                    nc.sync.dma_start(out=xt[:], in_=x_t[128 * k:128 * (k + 1), :])
                    xts.append(xt)
                    wt = wp.tile([128, KVLR + ROPE], dt, name=f"wAkv_{k}")
                    nc.scalar.dma_start(out=wt[:], in_=wa_kv[128 * k:128 * (k + 1), :])
                    wkv_res.append(wt)
                wq_res = []
                for k in range(HCH):
                    wt = wp.tile([128, QLR], dt, name=f"wAq_{k}")
                    nc.gpsimd.dma_start(out=wt[:], in_=wa_q[128 * k:128 * (k + 1), :])
                    wq_res.append(wt)

                def a_chunk(wts, o, c0, width, tag):
                    ps = pp_a.tile([width, SL], f32, name=f"ps_{tag}_{o}", tag="a_ps", bufs=2)
                    for hc in range(HCH):
                        nc.tensor.matmul(ps[:], wts[hc][:, c0:c0 + width], xts[hc][:],
                                         start=(hc == 0), stop=(hc == HCH - 1))
                    return ps

                # ---- kv chunks (rms-scaled locally) + k_pe (raw) ----
                ssq_kv = pp1_a.tile([1, SL], f32, name="ssq_kv")
                kv_sb = []
                for o in range(KCH):
                    ps = a_chunk(wkv_res, o, 128 * o, 128, "kv")
                    sb = rp_a.tile([128, SL], f32, name=f"kv_sb{o}")
                    nc.vector.tensor_copy(sb[:], ps[:])
                    kv_sb.append(sb)
                    sq = tp.tile([128, SL], dt, name=f"sqk{o}", tag="sq", bufs=2)
                    nc.scalar.activation(sq[:], ps[:], AF.Square)
                    nc.tensor.matmul(ssq_kv[:], ocol[:], sq[:], start=(o == 0), stop=(o == KCH - 1))
                ps_pe = a_chunk(wkv_res, 0, KVLR, ROPE, "pe")
                pe_sb = tp.tile([ROPE, SL], dt, name="pe_sb", tag="pe_sb", bufs=1)
                nc.vector.tensor_copy(pe_sb[:], ps_pe[:])
                nc.sync.dma_start(out=ag_in_kv[KCH * 128:KCH * 128 + ROPE, :], in_=pe_sb[:])
                ms_kv = tp.tile([1, SL], f32, name="ms_kv", tag="ms", bufs=2)
                nc.scalar.activation(ms_kv[:], ssq_kv[:], AF.Sqrt, scale=1.0 / KVLR)
                rkv = tp.tile([1, SL], f32r, name="rkv", tag="rr", bufs=2)
                with nc.allow_low_precision(reason="f32r rounding of rms scale"):
                    nc.vector.reciprocal(rkv[:], ms_kv[:])
                bc_kv = pp1_a.tile([128, SL], f32, name="bc_kv")
                nc.tensor.matmul(bc_kv[:], orow[:], rkv[:], start=True, stop=True)
                for o in range(KCH):
                    sc = tp.tile([128, SL], dt, name=f"sck{o}", tag="sc", bufs=3)
                    nc.vector.tensor_mul(sc[:], kv_sb[o][:], bc_kv[:])
                    nc.sync.dma_start(out=ag_in_kv[128 * o:128 * (o + 1), :], in_=sc[:])

                nc.gpsimd.collective_compute(
                    "AllGather", ALU.bypass,
                    replica_groups=[list(range(NC_))],
                    ins=[ag_in_kv[:]], outs=[ag_out_kv[:]],
                )

                # ---- q chunks: raw + ssq row, one AllGather ----
                ssq_q = pp1_a.tile([1, SL], f32, name="ssq_q")
                for o in range(QCH):
                    ps = a_chunk(wq_res, o, 128 * o, 128, "q")
                    qa_c = tp.tile([128, SL], dt, name=f"qac{o}", tag="qa_c", bufs=3)
                    nc.vector.tensor_copy(qa_c[:], ps[:])
                    nc.sync.dma_start(out=ag_in_q[128 * o:128 * (o + 1), :], in_=qa_c[:])
                    sq = tp.tile([128, SL], dt, name=f"sqq{o}", tag="sq", bufs=2)
                    nc.scalar.activation(sq[:], ps[:], AF.Square)
                    nc.tensor.matmul(ssq_q[:], ocol[:], sq[:], start=(o == 0), stop=(o == QCH - 1))
                ssq_sb = tp.tile([1, SL], f32, name="ssq_sb", tag="ss_sb", bufs=2)
                nc.vector.tensor_copy(ssq_sb[:], ssq_q[:])
                agq_w = ag_in_q.rearrange("(a b) q -> a (b q)", b=2)
                nc.sync.dma_start(out=agq_w[(QCH * 128) // 2:(QCH * 128) // 2 + 1, :],
                                  in_=ssq_sb[:].bitcast(dt))

                nc.gpsimd.collective_compute(
                    "AllGather", ALU.bypass,
                    replica_groups=[list(range(NC_))],
                    ins=[ag_in_q[:]], outs=[ag_out_q[:]],
                )

            # gathered views: [row, rank, token]
            agkv_3 = ag_out_kv.rearrange("(r c) q -> c r q", r=NC_)
            agq_3 = ag_out_q.rearrange("(r c) q -> c r q", r=NC_)
            agq_r = ag_out_q.rearrange("(r c) q -> r (c q)", r=NC_)

            def load_gathered(t, src3, r0, r1):
                """t[p, 256r+q] = src3[r0+p, r, q] in one strided DMA."""
                nc.sync.dma_start(
                    out=t[:].rearrange("p (r q) -> p r q", r=NC_),
                    in_=src3[r0:r1])

            # ================= Stage B: head-parallel attention =================
            with tc.tile_pool(name="sb_res", bufs=1) as rp, \
                 tc.tile_pool(name="sb_tmp", bufs=2) as tp, \
                 tc.tile_pool(name="sb_pt", bufs=4) as ptp, \
                 tc.tile_pool(name="sb_wo", bufs=1) as wsp, \
                 tc.tile_pool(name="sb_ag", bufs=1) as agp:

                # weight preloads (gpsimd queue; scalar stays free for exp)
                wkk_t = []
                wkv_t = []
                for l in range(KCH):
                    t = rp.tile([128, HPC * NOPE], dt, name=f"wkk_t{l}")
                    nc.gpsimd.dma_start(out=t[:], in_=wkvb_k[128 * l:128 * (l + 1), :])
                    wkk_t.append(t)
                    t2 = rp.tile([128, HPC * VD], dt, name=f"wkv_t{l}")
                    nc.gpsimd.dma_start(out=t2[:], in_=wkvb_v[128 * l:128 * (l + 1), :])
                    wkv_t.append(t2)
                # wo preload (gpsimd queue, overlaps the AllGathers / attention)
                wo_t = []
                for c in range(HCH):
                    t = wsp.tile([128, HID], dt, name=f"wo_t{c}")
                    nc.gpsimd.dma_start(out=t[:], in_=wo[128 * c:128 * (c + 1), :])
                    wo_t.append(t)

                # ---- post-AG-kv: rms scale, k_pe rope, K^T, V ----
                kT = [rp.tile([128, S], dt, name=f"kT{h}") for h in range(HPC)]
                v_t = [rp.tile([128, HPC * VD], dt, name=f"v_t{kb}") for kb in range(NKB)]
                kpe_f = rp.tile([ROPE, S], dt, name="kpe_f")
                with tc.tile_pool(name="sb_ckv", bufs=1) as ckvp, \
                     tc.tile_pool(name="sb_eps", bufs=2, space="PSUM") as pp_e:
                    ckv_g = []
                    for j in range(KCH):
                        t = ckvp.tile([128, S], dt, name=f"ckv_g{j}")
                        load_gathered(t, agkv_3, 128 * j, 128 * (j + 1))
                        ckv_g.append(t)
                    with tc.tile_pool(name="sb_kpe", bufs=1) as kpp:
                        kpe_b = kpp.tile([ROPE, S], dt, name="kpe_b")
                        load_gathered(kpe_b, agkv_3, KCH * 128, KCH * 128 + ROPE)

                        # k_pe rope (local bf16 cos/sin tables)
                        krot = kpp.tile([ROPE, S], dt, name="krot")
                        nc.vector.tensor_scalar(out=krot[0:32, :], in0=kpe_b[32:64, :],
                                                scalar1=-1.0, scalar2=None, op0=ALU.mult)
                        nc.vector.tensor_copy(krot[32:64, :], kpe_b[0:32, :])
                        kro = kpp.tile([ROPE, S], dt, name="kro")
                        nc.vector.tensor_mul(kro[:], kpe_b[:], cos_g[:])
                        nc.vector.tensor_mul(krot[:], krot[:], sin_g[:])
                        nc.vector.tensor_add(kpe_f[:], kro[:], krot[:])

                    # K^T and V (both heads)
                    for h in range(HPC):
                        for kc in range(S // 512):
                            ps = pp_e.tile([128, 512], f32, name=f"kt_ps{h}_{kc}", tag="bc", bufs=2)
                            for l in range(KCH):
                                nc.tensor.matmul(ps[:], wkk_t[l][:, NOPE * h:NOPE * (h + 1)],
                                                 ckv_g[l][:, 512 * kc:512 * (kc + 1)],
                                                 start=(l == 0), stop=(l == KCH - 1))
                            nc.vector.tensor_copy(kT[h][:, 512 * kc:512 * (kc + 1)], ps[:])
                    for kb in range(NKB):
                        ps = pp_e.tile([128, HPC * VD], f32, name=f"v_ps{kb}", tag="bc", bufs=2)
                        for l in range(KCH):
                            nc.tensor.matmul(ps[:], ckv_g[l][:, 128 * kb:128 * (kb + 1)], wkv_t[l][:],
                                             start=(l == 0), stop=(l == KCH - 1))
                        nc.vector.tensor_copy(v_t[kb][:], ps[:])

                # ---- q projections + attention h0, interleaved per panel ----
                with tc.tile_pool(name="sb_qps", bufs=1, space="PSUM") as qpp, \
                     tc.tile_pool(name="sb_rq", bufs=1, space="PSUM") as rqp, \
                     tc.tile_pool(name="sb_sc", bufs=2, space="PSUM") as scp, \
                     tc.tile_pool(name="sb_at", bufs=2, space="PSUM") as atp, \
                     tc.tile_pool(name="sb_sum", bufs=1, space="PSUM") as sup, \
                     tc.tile_pool(name="sb_qa", bufs=2) as qap:

                    wqb_t = []
                    for l in range(QCH):
                        t = qap.tile([128, HPC * 256], dt, name=f"wqb_t{l}",
                                     tag=f"wqb{l}", bufs=1)
                        nc.gpsimd.dma_start(out=t[:], in_=wqb[128 * l:128 * (l + 1), :])
                        wqb_t.append(t)

                    # q rms scale row (scalar queue so the sync queue's qa
                    # panel loads aren't blocked behind it)
                    ssq8_b = qap.tile([NC_, 2 * SL], dt, name="ssq8_b", tag="ssq8", bufs=1)
                    nc.scalar.dma_start(out=ssq8_b[:],
                                        in_=agq_r[:, QCH * 128 * SL:QROWS * SL])
                    ms8 = qap.tile([NC_, SL], f32, name="ms8", tag="ms8", bufs=1)
                    nc.scalar.activation(ms8[:], ssq8_b[:].bitcast(f32), AF.Sqrt,
                                         scale=1.0 / QLR)
                    rq8_t = qap.tile([NC_, SL], f32r, name="rq8_t", tag="rq8", bufs=1)
                    with nc.allow_low_precision(reason="f32r rounding of rms scale"):
                        nc.vector.reciprocal(rq8_t[:], ms8[:])
                    rq8 = rq8_t[:]

                    qn_sb = {}
                    qp_sb = {}

                    def q_panel(p, h):
                        """q projections for (head h, panel p); rms scale folded."""
                        qs = slice(PANEL * p, PANEL * (p + 1))
                        if h == 0:
                            qa_p = []
                            for l in range(QCH):
                                t = qap.tile([128, PANEL], dt, name=f"qa_p{p}_{l}",
                                             tag=f"qa_p{l}", bufs=2)
                                nc.sync.dma_start(
                                    out=t[:].rearrange("p (r q) -> p r q", r=2),
                                    in_=agq_3[128 * l:128 * (l + 1), 2 * p:2 * p + 2])
                                qa_p.append(t)
                            q_panel.qa = qa_p
                        qa_p = q_panel.qa
                        hcol = 256 * h
                        ps_qn = qpp.tile([128, PANEL], f32, name=f"qn_ps{h}_{p}", tag="qn", bufs=1)
                        for l in range(QCH):
                            nc.tensor.matmul(ps_qn[:], wqb_t[l][:, hcol:hcol + NOPE], qa_p[l][:],
                                             start=(l == 0), stop=(l == QCH - 1))
                        ps_qr = qpp.tile([128, PANEL], f32, name=f"qr_ps{h}_{p}", tag="qr", bufs=1)
                        for l in range(QCH):
                            nc.tensor.matmul(ps_qr[:], wqb_t[l][:, hcol + NOPE:hcol + 256], qa_p[l][:],
                                             start=(l == 0), stop=(l == QCH - 1))
                        if h == 0:
                            rqbc = rqp.tile([128, PANEL], f32, name=f"rqbc{p}", tag="rqbc", bufs=1)
                            nc.tensor.matmul(rqbc[:, 0:SL], sel[:, 128 * (2 * p):128 * (2 * p + 1)],
                                             rq8, start=True, stop=True)
                            nc.tensor.matmul(rqbc[:, SL:PANEL], sel[:, 128 * (2 * p + 1):128 * (2 * p + 2)],
                                             rq8, start=True, stop=True)
                            rqbc_sb = tp.tile([128, PANEL], f32, name=f"rqbc_sb{p}", tag="rqbc_sb", bufs=2)
                            nc.vector.tensor_copy(rqbc_sb[:], rqbc[:])
                            q_panel.rq = rqbc_sb
                            cos_s = tp.tile([ROPE, PANEL], dt, name=f"cos_s{p}", tag="cos_s", bufs=2)
                            nc.vector.tensor_mul(cos_s[:], cos_g[:, qs], rqbc_sb[0:ROPE, :])
                            sin_s = tp.tile([ROPE, PANEL], dt, name=f"sin_s{p}", tag="sin_s", bufs=2)
                            nc.vector.tensor_mul(sin_s[:], sin_g[:, qs], rqbc_sb[0:ROPE, :])
                            q_panel.cs = (cos_s, sin_s)
                        rqbc_sb = q_panel.rq
                        cos_s, sin_s = q_panel.cs
                        qn = rp.tile([128, PANEL], dt, name=f"qn_sb{h}_{p}")
                        nc.vector.tensor_mul(qn[:], ps_qn[:], rqbc_sb[:])
                        qn_sb[(h, p)] = qn
                        qt1 = tp.tile([ROPE, PANEL], f32, name=f"qt1_{h}_{p}", tag="qt1", bufs=1)
                        nc.vector.tensor_mul(qt1[:], ps_qr[0:ROPE, :], cos_s[:])
                        qt2 = tp.tile([ROPE, PANEL], f32, name=f"qt2_{h}_{p}", tag="qt2", bufs=1)
                        nc.vector.tensor_mul(qt2[:], ps_qr[ROPE:2 * ROPE, :], sin_s[:])
                        qp = rp.tile([ROPE, PANEL], dt, name=f"qp_sb{h}_{p}")
                        nc.vector.tensor_add(qp[:], qt1[:], qt2[:])
                        qp_sb[(h, p)] = qp

                    a2a_vx = [a2a_in[h].rearrange("(j x) q -> x j q", x=SHARD) for h in range(HPC)]
                    a2a_w = [a2a_in[h].rearrange("(a b) q -> a (b q)", b=2) for h in range(HPC)]
                    a2a_wv = [a2a_w[h].rearrange("(j y) q -> y j q", y=SHARD // 2) for h in range(HPC)]

                    def attn_panel(h, p):
                        """causal attention for (head h, panel p), unnormalized."""
                        nkb = 4 * (p + 1)
                        ps_at = atp.tile([128, PANEL], f32, name=f"at_ps{h}_{p}", tag="at_ps", bufs=2)
                        ps_sum = sup.tile([1, PANEL], f32, name=f"sum_ps{h}_{p}", tag="sum_ps", bufs=1)
                        pts = {}
                        np_ = nkb // 2  # sum pairs

                        def consume(kb):
                            nc.tensor.matmul(ps_at[:], v_t[kb][:, VD * h:VD * (h + 1)], pts[kb][:],
                                             start=(kb == 0), stop=(kb == nkb - 1))
                            if kb % 2 == 1:
                                pr = kb // 2
                                padd = tp.tile([128, PANEL], dt, name=f"padd{h}_{p}_{kb}",
                                               tag="padd", bufs=2)
                                nc.vector.tensor_add(padd[:], pts[kb - 1][:], pts[kb][:])
                                nc.tensor.matmul(ps_sum[:], ocol[:], padd[:],
                                                 start=(pr == 0), stop=(pr == np_ - 1))

                        for kb in range(nkb):
                            j = kb - 4 * p  # >= 0 on diagonal blocks
                            c0 = 128 * j if j > 0 else 0
                            ps_sc = scp.tile([128, PANEL], f32, name=f"sc_ps{h}_{p}_{kb}",
                                             tag="sc_ps", bufs=2)
                            nc.tensor.matmul(ps_sc[:, c0:], kT[h][:, 128 * kb:128 * (kb + 1)],
                                             qn_sb[(h, p)][:, c0:], start=True, stop=False)
                            nc.tensor.matmul(ps_sc[:, c0:], kpe_f[:, 128 * kb:128 * (kb + 1)],
                                             qp_sb[(h, p)][:, c0:], start=False, stop=True)
                            pt = ptp.tile([128, PANEL], dt, name=f"pt{h}_{p}_{kb}", tag="pt", bufs=4)
                            nc.scalar.activation(pt[:, c0:], ps_sc[:, c0:], AF.Exp, scale=SM_SCALE)
                            if j >= 0:
                                nc.gpsimd.affine_select(
                                    out=pt[:], in_=pt[:],
                                    pattern=[[1, PANEL]],
                                    compare_op=ALU.is_ge,
                                    fill=0.0,
                                    base=-128 * j,
                                    channel_multiplier=-1)
                            pts[kb] = pt
                            if kb > 0:
                                consume(kb - 1)
                        consume(nkb - 1)
                        at_b = tp.tile([128, PANEL], dt, name=f"at_b{h}_{p}", tag="at_b", bufs=2)
                        nc.vector.tensor_copy(at_b[:], ps_at[:])
                        sum_sb = tp.tile([1, PANEL], f32, name=f"sum_sb{h}_{p}", tag="sum_sb", bufs=2)
                        nc.vector.tensor_copy(sum_sb[:], ps_sum[:])
                        nc.sync.dma_start(
                            out=a2a_vx[h][0:VD, 2 * p:2 * p + 2],
                            in_=at_b[:].rearrange("p (r q) -> p r q", r=2))
                        nc.sync.dma_start(
                            out=a2a_wv[h][VD // 2:VD // 2 + 1, 2 * p:2 * p + 2],
                            in_=sum_sb[:].bitcast(dt).rearrange("p (r q) -> p r q", r=2))

                    for p in range(NPANEL):
                        q_panel(p, 0)
                        q_panel(p, 1)
                        attn_panel(0, p)
                    nc.gpsimd.collective_compute(
                        "AllToAll", ALU.bypass,
                        replica_groups=[list(range(NC_))],
                        ins=[a2a_in[0][:]], outs=[a2a_out[0][:]],
                    )
                    for p in range(NPANEL):
                        attn_panel(1, p)
                    nc.gpsimd.collective_compute(
                        "AllToAll", ALU.bypass,
                        replica_groups=[list(range(NC_))],
                        ins=[a2a_in[1][:]], outs=[a2a_out[1][:]],
                    )

                    # ---- post-exchange normalization + two-pass Wo ----
                    osb_stash = {}
                    for h in range(HPC):
                        a2a_ox = a2a_out[h].rearrange("(j x) q -> x j q", x=SHARD)
                        a2a_oj = a2a_out[h].rearrange("(j x) q -> j (x q)", x=SHARD)
                        att_all = agp.tile([128, NC_ * SL], dt, name=f"att_all{h}")
                        nc.sync.dma_start(out=att_all[:].rearrange("p (r q) -> p r q", r=NC_),
                                          in_=a2a_ox[0:VD])
                        sums_b = agp.tile([NC_, 2 * SL], dt, name=f"sums_b{h}")
                        nc.sync.dma_start(out=sums_b[:], in_=a2a_oj[:, VD * SL:SHARD * SL])
                        rec8 = agp.tile([NC_, SL], f32r, name=f"rec8{h}")
                        with nc.allow_low_precision(reason="f32r rounding of softmax recip"):
                            nc.vector.reciprocal(rec8[:], sums_b[:].bitcast(f32))
                        for m in range(NC_):
                            bc = atp.tile([128, SL], f32, name=f"nbc{h}_{m}", tag="at_ps", bufs=2)
                            nc.tensor.matmul(bc[:], sel[:, 128 * m:128 * (m + 1)], rec8[:],
                                             start=True, stop=True)
                            nc.vector.tensor_mul(att_all[:, SL * m:SL * (m + 1)],
                                                 att_all[:, SL * m:SL * (m + 1)], bc[:])
                        # Wo pass h: accumulate this parity's 8 head-chunks
                        for col in range(HID // 512):
                            for qb in range(SL // 128):
                                ps = qpp.tile([128, 512], f32, name=f"o_ps{h}_{col}_{qb}",
                                              tag=("qn" if (col + qb) % 2 == 0 else "qr"), bufs=1)
                                for m in range(NC_):
                                    nc.tensor.matmul(ps[:], att_all[:, SL * m + 128 * qb:SL * m + 128 * (qb + 1)],
                                                     wo_t[2 * m + h][:, 512 * col:512 * (col + 1)],
                                                     start=(m == 0), stop=(m == NC_ - 1))
                                if h == 0:
                                    st = wsp.tile([128, 512], dt, name=f"ost{col}_{qb}")
                                    nc.vector.tensor_copy(st[:], ps[:])
                                    osb_stash[(col, qb)] = st
                                else:
                                    osb = tp.tile([128, 512], f32, name=f"osb{col}_{qb}", tag="osb", bufs=2)
                                    nc.vector.tensor_add(osb[:], ps[:], osb_stash[(col, qb)][:])
                                    nc.sync.dma_start(out=out_loc[128 * qb:128 * (qb + 1), 512 * col:512 * (col + 1)],
                                                      in_=osb[:])

    nc.compile()
    return nc


def _to_dt(a, dt):
    if dt == bf16:
        return np.ascontiguousarray(a.astype(ml_dtypes.bfloat16))
    return np.ascontiguousarray(a.astype(np.float32))


def _prepare_inputs(dt, hidden_states, position_ids, Wqa, qa_ln_w, Wqb, Wkva, kv_ln_w, Wkvb, Wo):
    perm = np.concatenate([np.arange(0, ROPE, 2), np.arange(1, ROPE, 2)])
    X = np.asarray(hidden_states, np.float32).reshape(S, HID)
    Wqa = np.asarray(Wqa, np.float32)
    Wkva = np.asarray(Wkva, np.float32)
    wa_kv = np.concatenate([Wkva[:, :KVLR], Wkva[:, KVLR:][:, perm]], axis=1)  # (2048, 576)
    wqb_base = np.asarray(Wqb, np.float32) * np.asarray(qa_ln_w, np.float32)[:, None]
    wkvb_base = np.asarray(Wkvb, np.float32) * np.asarray(kv_ln_w, np.float32)[:, None]
    Wo = np.asarray(Wo, np.float32)

    head_blocks = []
    for h in range(NH):
        cols = wqb_base[:, 192 * h:192 * (h + 1)]
        nope = cols[:, :NOPE]
        pe_d = cols[:, NOPE:][:, perm]
        rot = np.concatenate([-pe_d[:, 32:], pe_d[:, :32]], axis=1)
        head_blocks.append(np.concatenate([nope, pe_d, rot], axis=1))  # (1536, 256)
    k_blocks = [wkvb_base[:, 256 * h:256 * h + NOPE] for h in range(NH)]
    v_blocks = [wkvb_base[:, 256 * h + NOPE:256 * (h + 1)] for h in range(NH)]

    # host-precomputed rope tables (deinterleaved layout), [64, S] transposed
    inv = (1.0 / (THETA ** (np.arange(0, ROPE, 2, dtype=np.float64) / ROPE)))
    pos = np.asarray(position_ids).reshape(-1).astype(np.float64)
    emb = np.concatenate([np.outer(pos, inv), np.outer(pos, inv)], axis=1)  # (S, 64)
    cos_np = _to_dt(np.cos(emb).T.astype(np.float32), dt)
    sin_np = _to_dt(np.sin(emb).T.astype(np.float32), dt)

    wakv_d = _to_dt(wa_kv, dt)
    waq_d = _to_dt(Wqa, dt)
    wo_d = _to_dt(Wo, dt)
    ones_col_d = _to_dt(np.ones((128, 1), np.float32), dt)
    sel_np = np.zeros((NC_, NC_ * 128), np.float32)
    for m in range(NC_):
        sel_np[m, 128 * m:128 * (m + 1)] = 1.0

    in_maps = []
    for c in range(NC_):
        rows = slice(SL * c, SL * (c + 1))
        in_maps.append({
            "x_t": _to_dt(X[rows, :].T, dt),
            "ones_row": np.ones((1, 128), np.float32),
            "wa_kv": wakv_d,
            "wa_q": waq_d,
            "wqb": _to_dt(np.concatenate([head_blocks[HPC * c + h] for h in range(HPC)], axis=1), dt),
            "wkvb_k": _to_dt(np.concatenate([k_blocks[HPC * c + h] for h in range(HPC)], axis=1), dt),
            "wkvb_v": _to_dt(np.concatenate([v_blocks[HPC * c + h] for h in range(HPC)], axis=1), dt),
            "wo": wo_d,
            "ones_col": ones_col_d,
            "sel_in": sel_np,
            "cos_in": cos_np,
            "sin_in": sin_np,
        })
    return in_maps


def run(inputs, trace=False, trace_cores=None, dt=None):
    dt = dt if dt is not None else DT
    key = ("nc", str(dt))
    if key not in _CACHE:
        _CACHE[key] = build_program(dt)
    nc = _CACHE[key]
    in_maps = _prepare_inputs(dt, **inputs)
    res = run_bass_kernel_spmd(nc, in_maps, list(range(NC_)), trace=trace,
                               trace_cores=trace_cores)
    out = np.concatenate([res.results[c]["out_loc"] for c in range(NC_)], axis=0)
    return out.reshape(1, S, HID), res


def kernel(**inputs) -> np.ndarray:
    out, _ = run(inputs, trace=False)
    return out
